# revision 56
# baseline (speedup 1.0000x reference)
"""Trainium2 Bass kernel for nn_Block_29085518528833 (PVT-style pooling
attention block + IRB conv-MLP).

Sharding: 8 cores = 4 batches x 2 token-halves. Each core processes one
batch's full image for the (tiny, replicated) pooling/kv path, and a
4992-token range (4608 own + 384 halo) for the token-parallel paths.
The host permutes tokens so every core's own range is rows [0, 4992) --
a single uniform SPMD program, no cross-core communication.

v2 (from v1 trace analysis: PE spent 80% of time at K=4/8 half clock,
fp32 matmuls ran in 4-cyc/row HIGH mode, 494 rank-1 bias matmuls and 27
activation-table reloads):
 - every PE operand is bf16 or fp8 (no fp32/fp32r matmuls anywhere)
 - rsqrt for LN = exp(-0.5*ln(v+eps)) so the whole kernel uses one
   activation table set (exp/ln/relu/identity/copy) -- zero reloads
 - proj bias rides the softmax denominator row: A is normalized with 65
   rows (row 64 == 1) and projT gets a 65th row = proj_b/8
 - fc1/dwconv biases fold into the hardswish Relu's per-partition bias
   port; the multiplicand side is reconstructed as w = 256*c1p - 8 on
   DVE, so all fc1/dw rank-1 bias matmuls are gone
 - the fc1->dwconv ring is one flat [128, 16, 54, 98] fp8 buffer
   covering all 52 rows + zero pad rows, so fc1 writes once (no
   neighbour-slot boundary copies) and dwconv reads are uniform for all
   groups (pads make edge clipping unnecessary)
 - hardswish outputs are written to the ring/t2 directly as fp8 by DVE
"""

import os
import sys

sys.path.insert(0, "/opt/trn_rl_repo")

from contextlib import ExitStack

import numpy as np
import ml_dtypes

import concourse.bass as bass
import concourse.bacc as bacc
import concourse.mybir as mybir
from concourse.tile import TileContext

FP = mybir.dt.float32
BF = mybir.dt.bfloat16
F8 = mybir.dt.float8e4
AF = mybir.ActivationFunctionType
ALU = mybir.AluOpType
DR = mybir.MatmulPerfMode.DoubleRow

B = 4
C = 512
NH = 8
HD = 64
HID = 2048
HIMG = 96
NTOK = HIMG * HIMG
EPS = 1e-5
OHS = [8, 6, 5, 4]
LS = [o * o for o in OHS]
LOFF = [0, 64, 100, 125]
L = 141
AREAS = [144, 256, 400, 576]
L32 = [64, 64, 32, 32]  # LS padded to 32-multiples

HALF = NTOK // 2
HALO = 384
TRNG = HALF + HALO            # 4992
GRP = 384
NGRP = TRNG // GRP            # 13
NROWS = TRNG // HIMG          # 52
CCH = C // 128                # 4
MCH = HID // 128              # 16
NIMG_TILES = NTOK // 128      # 72
SCALE = HD ** (-0.5)
W16 = 16.0                    # fp8 weight pre-scale
RW = 98                       # padded row width in the t ring (1+96+1)
NRING = NROWS + 2             # 54 ring rows (1 zero pad row each side)

TAPS = [(di, dj) for di in (-1, 0, 1) for dj in (-1, 0, 1)]

# sim-compatible mode: the interpreter only supports 3D DoubleRow rhs APs,
# so split the dwconv pair matmuls per row when simulating.
DW3D = os.environ.get("DW3D", "0") == "1"

_cache = {}


LOFF176 = [0, 64, 128, 160]


def _build_masks():
    M = np.zeros((NTOK, 176), np.float32)
    for s, oh in enumerate(OHS):
        sh = (np.arange(oh) * HIMG) // oh
        eh = -((-(np.arange(oh) + 1) * HIMG) // oh)
        for i in range(oh):
            for j in range(oh):
                hmask = np.zeros(HIMG, bool)
                hmask[sh[i]:eh[i]] = True
                wmask = np.zeros(HIMG, bool)
                wmask[sh[j]:eh[j]] = True
                tok = (hmask[:, None] & wmask[None, :]).reshape(-1)
                M[tok, LOFF176[s] + i * oh + j] = 1.0
    return M


def build_program():
    nc = bacc.Bacc("TRN2", target_bir_lowering=False, debug=False, num_devices=8)

    def din(name, shape, dtype=FP):
        return nc.dram_tensor(name, list(shape), dtype, kind="ExternalInput").ap()

    I = {}
    I["xbf"] = din("xbf", [NTOK, C], BF)
    I["masks"] = din("masks", [NTOK, 176], F8)
    I["qgwT"] = din("qgwT", [C, C], BF)
    I["qb"] = din("qb", [128, CCH])
    I["kwT"] = din("kwT", [C, C], BF)
    I["vwT"] = din("vwT", [C, C], BF)
    I["projT"] = din("projT", [NH, 65, C], BF)
    I["fc1p"] = din("fc1p", [2, 128, 2, HID], F8)
    I["diagp"] = din("diagp", [128, MCH, 3, 2, 128], F8)
    I["diags"] = din("diags", [128, MCH, 128], F8)
    I["diagv"] = din("diagv", [128, MCH, 2, 128], F8)
    I["fc2p"] = din("fc2p", [MCH // 2, 128, 2, C], F8)
    I["fc2b"] = din("fc2b", [1, C], BF)
    I["fc1bP"] = din("fc1bP", [128, MCH])
    I["convbP"] = din("convbP", [128, MCH])
    I["g1rep"] = din("g1rep", [128, C], BF)
    I["abrep"] = din("abrep", [128, 4, C], BF)
    I["agrep"] = din("agrep", [128, C], BF)
    I["btrep"] = din("btrep", [128, C], BF)
    I["poolw"] = din("poolw", [128, CCH, 4, 9])
    I["poolabd"] = din("poolabd", [128, CCH, 4])
    I["v2init"] = din("v2init", [L, NH * 128], BF)
    I["identb"] = din("identb", [128, 128], BF)

    out_dram = nc.dram_tensor("out", [TRNG, C], BF, kind="ExternalOutput").ap()

    with TileContext(nc) as tc:
        _program(nc, tc, I, out_dram)
    nc.compile()
    return nc


def _program(nc, tc, I, out_dram):
    ctx = ExitStack()
    with ctx:
        consts = ctx.enter_context(tc.tile_pool(name="consts", bufs=1))
        persist = ctx.enter_context(tc.tile_pool(name="persist", bufs=1))
        small = ctx.enter_context(tc.tile_pool(name="small", bufs=4))

        identb = consts.tile([128, 128], BF)
        nc.sync.dma_start(out=identb[:], in_=I["identb"])
        eps_t = consts.tile([128, 1], FP)
        nc.vector.memset(eps_t, EPS)
        onesP = consts.tile([128, 128], BF)
        nc.vector.memset(onesP, 1.0)

        # ---- C/E resident weights (~46KB/partition of DMAs).  Emitted
        # after phase A so the first x/mask streaming loads aren't queued
        # behind 6MB of weight traffic; they still land long before phase C.
        wCE = ctx.enter_context(tc.tile_pool(name="wCE", bufs=1))

        def load_wCE():
            projT8 = []
            for h in range(NH):
                t = wCE.tile([65, C], BF, name=f"projT{h}")
                nc.sync.dma_start(out=t[:], in_=I["projT"][h])
                projT8.append(t)
            fc1p = [wCE.tile([128, 2, HID], F8, name=f"fc1p{j}")
                    for j in range(2)]
            for j in range(2):
                nc.sync.dma_start(out=fc1p[j][:], in_=I["fc1p"][j])
            diagp = wCE.tile([128, MCH, 3, 2, 128], F8)
            nc.sync.dma_start(out=diagp[:], in_=I["diagp"])
            diags = wCE.tile([128, MCH, 128], F8)
            nc.sync.dma_start(out=diags[:], in_=I["diags"])
            diagv = wCE.tile([128, MCH, 2, 128], F8)
            nc.sync.dma_start(out=diagv[:], in_=I["diagv"])
            fc2p = [wCE.tile([128, 2, C], F8, name=f"fc2p{j}")
                    for j in range(MCH // 2)]
            for j in range(MCH // 2):
                nc.sync.dma_start(out=fc2p[j][:], in_=I["fc2p"][j])
            fc2b_sb = wCE.tile([1, C], BF)
            nc.sync.dma_start(out=fc2b_sb[:], in_=I["fc2b"])
            fc1bP = wCE.tile([128, MCH], FP)
            nc.sync.dma_start(out=fc1bP[:], in_=I["fc1bP"])
            convbP = wCE.tile([128, MCH], FP)
            nc.sync.dma_start(out=convbP[:], in_=I["convbP"])
            return projT8, fc1p, diagp, diags, diagv, fc2p, fc2b_sb, fc1bP, convbP

        den4 = [persist.tile([128, GRP], BF, name=f"den4_{j}") for j in range(2)]
        for j in range(2):
            nc.vector.memset(den4[j][:], 1.0)

        qT_sb = [persist.tile([128, TRNG], BF, name=f"qT{m}") for m in range(CCH)]
        kT_sb = [persist.tile([128, L], BF, name=f"kT{m}") for m in range(CCH)]
        V_a = persist.tile([128, NH * 128], BF)
        V_b = persist.tile([13, NH * 128], BF)
        nc.sync.dma_start(out=V_a[:], in_=I["v2init"][0:128, :])
        nc.sync.dma_start(out=V_b[:], in_=I["v2init"][128:L, :])
        qb_sb = persist.tile([128, CCH], FP)
        nc.sync.dma_start(out=qb_sb[:], in_=I["qb"])

        # fc1 -> dwconv ring: flat fp8 buffer covering all rows.
        # ring row r+1 holds image row r; rows 0 and NRING-1 and cols 0 and
        # RW-1 are zero pad (written once via DMA from zpad).
        ring = persist.tile([128, MCH, NRING, RW], F8, name="ring")
        ring_ap = ring[:]
        pstr_ring = ring_ap.ap[0][0]
        # zero pads via DVE memsets -- pad DMAs were ~100k one-byte
        # descriptors that stalled the input queues for ~190us at start.
        nc.vector.memset(ring[:, :, 0, :], 0.0)
        nc.vector.memset(ring[:, :, NRING - 1, :], 0.0)
        nc.vector.memset(ring[:, :, :, 0], 0.0)
        nc.vector.memset(ring[:, :, :, RW - 1], 0.0)

        def ln_factors(xt, p):
            stats = small.tile([128, 6], FP, name="stats", tag="stats")
            nc.vector.bn_stats(out=stats[:p, :], in_=xt)
            mv = small.tile([128, 2], FP, name="mv", tag="mv")
            nc.vector.bn_aggr(out=mv[:p, :], in_=stats[:p, :])
            # Sqrt sites are clustered between exp batches, so the act-table
            # switch costs ~2 loads/group (Ln+Exp pairing measured far worse:
            # the table picker alternates sets on every ln<->exp transition).
            sd = small.tile([128, 1], FP, name="sd", tag="sd")
            nc.scalar.activation(out=sd[:p], in_=mv[:p, 1:2], func=AF.Sqrt,
                                 bias=eps_t[:p], scale=1.0)
            rs = small.tile([128, 1], FP, name="rs", tag="rs")
            nc.vector.reciprocal(rs[:p], sd[:p])
            nmurs = small.tile([128, 1], FP, name="nmurs", tag="nmurs")
            nc.vector.tensor_scalar(out=nmurs[:p], in0=mv[:p, 0:1],
                                    scalar1=rs[:p], scalar2=-1.0,
                                    op0=ALU.mult, op1=ALU.mult)
            return rs, nmurs

        phAB = ctx.enter_context(ExitStack())
        spool = phAB.enter_context(tc.tile_pool(name="spool", bufs=1))

        # ============ PHASE A: LN1, pool sums, q^T ============
        s_sb = []
        with ExitStack() as phA:
            strA = phA.enter_context(tc.tile_pool(name="strA", bufs=3))
            strX = phA.enter_context(tc.tile_pool(name="strX", bufs=2))
            wA = phA.enter_context(tc.tile_pool(name="wA", bufs=1))
            qgwT = [wA.tile([128, C], BF, name=f"qgwT{m}") for m in range(CCH)]
            for m in range(CCH):
                nc.sync.dma_start(out=qgwT[m][:],
                                  in_=I["qgwT"][m * 128:(m + 1) * 128, :])
            psA = phA.enter_context(tc.tile_pool(name="psA", bufs=1, space="PSUM"))
            spsA = psA.tile([128, C], FP, name="spsA")
            spsB = psA.tile([48, C], FP, name="spsB")
            # scale slices within the two banks (32-aligned bases)
            sps = [spsA[0:64], spsA[64:100], spsB[0:25], spsB[32:48]]
            psT = phA.enter_context(tc.tile_pool(name="psT", bufs=3, space="PSUM"))
            psQ = phA.enter_context(tc.tile_pool(name="psQ", bufs=3, space="PSUM"))
            xTg = phA.enter_context(tc.tile_pool(name="xTg", bufs=2))

            xT_cur = None
            for ti in range(NIMG_TILES):
                if ti % 3 == 0:
                    # batched loads: 3 token-tiles per DMA (bf16: DVE/ACT
                    # compute in fp32 internally, bf16 input rounding is
                    # harmless for LN stats)
                    xt3 = strX.tile([128, 3, C], BF, name="xt3", tag="xt")
                    nc.sync.dma_start(
                        out=xt3[:],
                        in_=I["xbf"][ti * 128:(ti + 3) * 128, :].rearrange(
                            "(t p) c -> p t c", p=128))
                    mt3 = strA.tile([128, 3, 176], F8, name="mt3", tag="mt")
                    nc.sync.dma_start(
                        out=mt3[:],
                        in_=I["masks"][ti * 128:(ti + 3) * 128, :].rearrange(
                            "(t p) c -> p t c", p=128))
                sub3 = ti % 3
                xt = xt3[:, sub3, :]
                rs, nmurs = ln_factors(xt, 128)
                xh = strA.tile([128, C], BF, name="xh", tag="xh")
                nc.scalar.activation(out=xh[:], in_=xt, func=AF.Identity,
                                     bias=nmurs[:], scale=rs[:])
                nc.tensor.matmul(spsA[:], mt3[:, sub3, 0:128], xh[:],
                                 start=(ti == 0), stop=(ti == NIMG_TILES - 1))
                nc.tensor.matmul(spsB[:], mt3[:, sub3, 128:176], xh[:],
                                 start=(ti == 0), stop=(ti == NIMG_TILES - 1))
                if ti < TRNG // 128:
                    gi, sub = divmod(ti, 3)
                    if sub == 0:
                        xT_cur = xTg.tile([128, CCH, GRP], BF, name="xT",
                                          tag="xT")
                    tp = psT.tile([128, 4, 128], BF, name="tpA", tag="tpA")
                    for cc in range(CCH):
                        nc.tensor.transpose(tp[:, cc, :],
                                            xh[:, cc * 128:(cc + 1) * 128],
                                            identb[:])
                    nc.vector.tensor_copy(
                        xT_cur[:, :, sub * 128:(sub + 1) * 128], tp[:])
                    if sub == 2:
                        for m in range(CCH):
                            qp = psQ.tile([128, GRP], FP, name="qp", tag="qp")
                            for cc in range(CCH):
                                nc.tensor.matmul(qp[:], qgwT[cc][:, m * 128:(m + 1) * 128],
                                                 xT_cur[:, cc, :], start=(cc == 0),
                                                 stop=(cc == CCH - 1))
                            nc.scalar.activation(
                                out=qT_sb[m][:, gi * GRP:(gi + 1) * GRP], in_=qp[:],
                                func=AF.Identity, bias=qb_sb[:, m:m + 1], scale=1.0)

            SBASE = [0, 64, 0, 32]
            for s in range(4):
                t = spool.tile([L32[s] + SBASE[s], C], BF, name=f"ssb{s}")
                nc.scalar.copy(t[SBASE[s]:SBASE[s] + LS[s], :], sps[s][:])
                s_sb.append(t)

        (projT8, fc1p, diagp, diags, diagv, fc2p, fc2b_sb, fc1bP,
         convbP) = load_wCE()

        # ============ PHASE B: pool dwconv + attn LN + k/v ============
        with ExitStack() as phB:
            wB = phB.enter_context(tc.tile_pool(name="wB", bufs=1))
            kwT = [wB.tile([128, C], BF, name=f"kwT{m}") for m in range(CCH)]
            vwT = [wB.tile([128, C], BF, name=f"vwT{m}") for m in range(CCH)]
            for m in range(CCH):
                nc.sync.dma_start(out=kwT[m][:],
                                  in_=I["kwT"][m * 128:(m + 1) * 128, :])
                nc.sync.dma_start(out=vwT[m][:],
                                  in_=I["vwT"][m * 128:(m + 1) * 128, :])
            g1rep = wB.tile([128, C], BF, name="g1rep")
            nc.sync.dma_start(out=g1rep[:], in_=I["g1rep"])
            abrep = wB.tile([128, 4, C], BF, name="abrep")
            nc.sync.dma_start(out=abrep[:], in_=I["abrep"])
            agrep = wB.tile([128, C], BF, name="agrep")
            nc.sync.dma_start(out=agrep[:], in_=I["agrep"])
            btrep = wB.tile([128, C], BF, name="btrep")
            nc.sync.dma_start(out=btrep[:], in_=I["btrep"])
            poolw = wB.tile([128, CCH, 4, 9], FP, name="poolw")
            nc.sync.dma_start(out=poolw[:], in_=I["poolw"])
            poolabd = wB.tile([128, CCH, 4], FP, name="poolabd")
            nc.sync.dma_start(out=poolabd[:], in_=I["poolabd"])

            psB = phB.enter_context(tc.tile_pool(name="psB", bufs=3, space="PSUM"))
            sbB = phB.enter_context(tc.tile_pool(name="sbB", bufs=2))
            accP = phB.enter_context(tc.tile_pool(name="accP", bufs=2))
            psK = phB.enter_context(tc.tile_pool(name="psK", bufs=1, space="PSUM"))
            kp4 = [psK.tile([128, 144], FP, name=f"kp4_{m}") for m in range(CCH)]

            rhskv = [wB.tile([128, 144], BF, name=f"rhskv{m}") for m in range(CCH)]
            for m in range(CCH):
                nc.vector.memset(rhskv[m][:, L:144], 0.0)

            SBASE = [0, 64, 0, 32]
            for s in range(4):
                b0 = SBASE[s]
                nc.vector.tensor_mul(s_sb[s][b0:b0 + LS[s], :],
                                     s_sb[s][b0:b0 + LS[s], :],
                                     g1rep[b0:b0 + LS[s], :])
                nc.vector.tensor_add(s_sb[s][b0:b0 + LS[s], :],
                                     s_sb[s][b0:b0 + LS[s], :],
                                     abrep[b0:b0 + LS[s], s, :])

            pn = []
            for s in range(4):
                oh = OHS[s]
                s1T = [sbB.tile([128, LS[s]], BF, name=f"s1T{s}_{cc}", tag=f"s1T{cc}")
                       for cc in range(CCH)]
                b0 = SBASE[s]
                for cc in range(CCH):
                    tp = psB.tile([128, 512], BF, name="tpB", tag="pb")
                    nc.tensor.transpose(tp[:, 0:L32[s]],
                                        s_sb[s][b0:b0 + L32[s],
                                                cc * 128:(cc + 1) * 128],
                                        identb[b0:b0 + L32[s], b0:b0 + L32[s]])
                    nc.scalar.copy(s1T[cc][:], tp[:, 0:LS[s]])
                acc = [accP.tile([128, L32[s]], BF, name=f"acc{s}_{cc}", tag=f"acc{cc}")
                       for cc in range(CCH)]
                for cc in range(CCH):
                    nc.vector.tensor_scalar_add(acc[cc][:, 0:LS[s]], s1T[cc][:],
                                                poolabd[:, cc, s:s + 1])
                    for tap, (di, dj) in enumerate(TAPS):
                        oi0 = max(0, -di)
                        oi1 = oh - max(0, di)
                        oj0 = max(0, -dj)
                        oj1 = oh - max(0, dj)
                        if oi1 <= oi0 or oj1 <= oj0:
                            continue
                        o_ap = acc[cc][:, 0:LS[s]].rearrange(
                            "p (i j) -> p i j", i=oh)[:, oi0:oi1, oj0:oj1]
                        i_ap = s1T[cc][:].rearrange("p (i j) -> p i j", i=oh)[
                            :, oi0 + di:oi1 + di, oj0 + dj:oj1 + dj]
                        nc.vector.scalar_tensor_tensor(
                            out=o_ap, in0=i_ap, scalar=poolw[:, cc, s, tap:tap + 1],
                            in1=o_ap, op0=ALU.mult, op1=ALU.add)
                q_s = sbB.tile([LS[s], C], BF, name=f"q_s{s}", tag="q_s")
                for cc in range(CCH):
                    tp = psB.tile([128, 512], BF, name="tpB2", tag="pb")
                    nc.tensor.transpose(tp[0:L32[s], 0:128], acc[cc][:], identb[:])
                    nc.scalar.copy(q_s[:, cc * 128:(cc + 1) * 128], tp[0:LS[s], 0:128])
                rs, nmurs = ln_factors(q_s[:], LS[s])
                pn_s = sbB.tile([L32[s], C], BF, name=f"pn{s}", tag="pn_s")
                nc.scalar.activation(out=pn_s[0:LS[s], :], in_=q_s[:],
                                     func=AF.Identity,
                                     bias=nmurs[0:LS[s]], scale=rs[0:LS[s]])
                nc.vector.tensor_mul(pn_s[0:LS[s], :], pn_s[0:LS[s], :],
                                     agrep[0:LS[s], :])
                nc.vector.tensor_add(pn_s[0:LS[s], :], pn_s[0:LS[s], :],
                                     btrep[0:LS[s], :])
                pn.append(pn_s)
                # transpose + K projection for this scale immediately, so
                # the K matmuls overlap the next scale's DVE pool chain
                # (each scale's kp column range is an independent psum
                # accumulation group).
                for cc in range(CCH):
                    tp = psB.tile([128, 512], BF, name="tpB3", tag="pb")
                    nc.tensor.transpose(tp[:, 0:L32[s]], pn_s[:, cc * 128:(cc + 1) * 128],
                                        identb[0:L32[s], 0:L32[s]])
                    nc.scalar.copy(rhskv[cc][:, LOFF[s]:LOFF[s] + LS[s]], tp[:, 0:LS[s]])
                for m in range(CCH):
                    for cc in range(CCH):
                        nc.tensor.matmul(kp4[m][:, LOFF[s]:LOFF[s] + LS[s]],
                                         kwT[cc][:, m * 128:(m + 1) * 128],
                                         rhskv[cc][:, LOFF[s]:LOFF[s] + LS[s]],
                                         start=(cc == 0), stop=(cc == CCH - 1))

            for m in range(CCH):
                nc.scalar.copy(kT_sb[m][:], kp4[m][:, 0:L])
            vp = psB.tile([128, C], FP, name="vp", tag="pb")
            for cc in range(CCH):
                nc.tensor.matmul(vp[:], rhskv[cc][:, 0:128], vwT[cc][:],
                                 start=(cc == 0), stop=(cc == CCH - 1))
            for h in range(NH):
                nc.scalar.copy(V_a[:, h * 128:h * 128 + 64],
                               vp[:, h * 64:h * 64 + 64])
            vp2 = psB.tile([13, C], FP, name="vp2", tag="pb")
            for cc in range(CCH):
                nc.tensor.matmul(vp2[:], rhskv[cc][:, 128:L], vwT[cc][:],
                                 start=(cc == 0), stop=(cc == CCH - 1))
            for h in range(NH):
                nc.scalar.copy(V_b[:, h * 128:h * 128 + 64],
                               vp2[:, h * 64:h * 64 + 64])

        phAB.close()

        # ============ PHASE C+E fused: attention + proj + LN2 + fc1 ->
        # ============ dwconv + fc2 + residual, pipelined over groups ====
        with ExitStack() as phC:
            strC = phC.enter_context(tc.tile_pool(name="strC", bufs=2))
            sbE = phC.enter_context(tc.tile_pool(name="sbE", bufs=2))
            sbA = phC.enter_context(tc.tile_pool(name="sbA", bufs=8))
            sbT = phC.enter_context(tc.tile_pool(name="sbT", bufs=2))
            strU = phC.enter_context(tc.tile_pool(name="strU", bufs=2))
            strE = phC.enter_context(tc.tile_pool(name="strE", bufs=2))
            ring_t2 = phC.enter_context(tc.tile_pool(name="ring_t2", bufs=8))
            dramP = phC.enter_context(tc.tile_pool(name="dramP", bufs=1,
                                                   space="DRAM"))
            # x2 (attn residual) is spilled to DRAM between LN2 and the
            # final residual add -- SBUF is too tight to keep 9 copies.
            x2d = dramP.tile([TRNG, C], BF, name="x2d")

            psC = phC.enter_context(tc.tile_pool(name="psC", bufs=3, space="PSUM"))
            psF2 = phC.enter_context(tc.tile_pool(name="psF2", bufs=2,
                                                  space="PSUM"))
            psO = phC.enter_context(tc.tile_pool(name="psO", bufs=1, space="PSUM"))

            t2_of = {}   # g -> [8 pair tiles]

            def emit_attn_group(g):
                g0 = g * GRP
                A_h = []
                for h in range(NH):
                    m, hh = h // 2, (h % 2) * 64
                    Sa = psC.tile([128, C], FP, name="Sa", tag="pc")
                    nc.tensor.matmul(Sa[:, 0:GRP], kT_sb[m][hh:hh + 64, 0:128],
                                     qT_sb[m][hh:hh + 64, g0:g0 + GRP],
                                     start=True, stop=True)
                    Sb = psC.tile([13, GRP], FP, name="Sb", tag="pc")
                    nc.tensor.matmul(Sb[:], kT_sb[m][hh:hh + 64, 128:L],
                                     qT_sb[m][hh:hh + 64, g0:g0 + GRP],
                                     start=True, stop=True)
                    Ea = sbE.tile([128, GRP], BF, name="Ea", tag="Ea")
                    nc.scalar.activation(out=Ea[:], in_=Sa[:, 0:GRP], func=AF.Exp)
                    Eb = sbE.tile([13, GRP], BF, name="Eb", tag="Ea")
                    nc.scalar.activation(out=Eb[:], in_=Sb[:], func=AF.Exp)
                    # U (rows 0..63) and den (row 64) in one accumulation
                    Uh = psC.tile([65, GRP], FP, name="Uh", tag="pc")
                    nc.tensor.matmul(Uh[:], V_a[:, h * 128:h * 128 + 65],
                                     Ea[:], start=True, stop=False)
                    nc.tensor.matmul(Uh[:], V_b[:, h * 128:h * 128 + 65],
                                     Eb[:], start=False, stop=True)
                    p0 = (h % 4) * 32
                    nc.scalar.copy(den4[h // 4][p0:p0 + 1, :], Uh[64:65, :])
                    Ah = sbA.tile([65, GRP], BF, name="Ah", tag="Ah")
                    nc.scalar.copy(Ah[:], Uh[0:65, :])
                    A_h.append(Ah)
                for j in range(2):
                    with nc.allow_low_precision("bf16 reciprocal feeds matmul"):
                        nc.vector.reciprocal(den4[j][:], den4[j][:])
                # normalize: broadcast each head's reciprocal over 65 rows
                # (row 64 becomes exactly 1, carrying proj_b via projT row 64)
                for h in range(NH):
                    p0 = (h % 4) * 32
                    rr = psC.tile([65, GRP], FP, name="rr", tag="pc")
                    nc.tensor.matmul(rr[:], onesP[p0:p0 + 1, 0:65],
                                     den4[h // 4][p0:p0 + 1, :],
                                     start=True, stop=True,
                                     tile_position=(p0, 0))
                    nc.vector.tensor_mul(A_h[h][:], A_h[h][:], rr[:])
                # proj + residual + LN2 + transposes
                xh2T = [sbT.tile([128, 2, GRP], F8, name=f"xh2T{j}", tag=f"xh2T{j}")
                        for j in range(2)]
                for sub in range(3):
                    r0 = g0 + sub * 128
                    xp = psC.tile([128, C], FP, name="xp", tag="pc")
                    for h in range(NH):
                        nc.tensor.matmul(xp[:], A_h[h][:, sub * 128:(sub + 1) * 128],
                                         projT8[h][:], start=(h == 0),
                                         stop=(h == NH - 1))
                    xt2 = strC.tile([128, C], BF, name="xt2", tag="xt2")
                    nc.sync.dma_start(out=xt2[:], in_=I["xbf"][r0:r0 + 128, :])
                    x2s = strC.tile([128, C], BF, name="x2s", tag="osb")
                    nc.vector.tensor_add(x2s[:], xt2[:], xp[:])
                    nc.sync.dma_start(out=x2d[r0:r0 + 128, :], in_=x2s[:])
                    rs, nmurs = ln_factors(x2s[:], 128)
                    xh2 = strC.tile([128, C], BF, name="xh2", tag="xh2")
                    nc.scalar.activation(out=xh2[:], in_=x2s[:], func=AF.Identity,
                                         bias=nmurs[:], scale=rs[:])
                    tp = psC.tile([128, C], BF, name="tpC", tag="pc")
                    for cc in range(CCH):
                        nc.tensor.transpose(tp[:, cc * 128:(cc + 1) * 128],
                                            xh2[:, cc * 128:(cc + 1) * 128],
                                            identb[:])
                    for j in range(2):
                        nc.scalar.copy(
                            xh2T[j][:, :, sub * 128:(sub + 1) * 128],
                            tp[:, j * 256:(j + 1) * 256].rearrange(
                                "p (two q) -> p two q", two=2))
                return xh2T

            def emit_fc1_pair(g, xh2T, mp):
                # fc1 for channel chunks 2mp, 2mp+1 (fp8 DoubleRow).  The fc1
                # bias rides the Relu's per-partition bias port:
                #   c1p  = relu(fp/256 + (B+8)/256)       (scalar, from PSUM)
                #   w1   = 256*c1p - 8    (= (16/6)y, DVE tensor_scalar)
                #   ring = min(c1p, 16/256) * w1 = hs(y)/6 (DVE STT, fp8 out)
                fp2 = psF2.tile([128, 2, C], FP, name="fp2", tag="pf")
                for i in range(2):
                    m = 2 * mp + i
                    for j in range(2):
                        nc.tensor.matmul(
                            fp2[:, i, 0:GRP],
                            fc1p[j][:, :, m * 128:(m + 1) * 128],
                            xh2T[j][:], start=(j == 0), stop=(j == 1),
                            perf_mode=DR)
                c1p = strU.tile([128, 2, GRP], BF, name="c1p", tag="c1p")
                for i in range(2):
                    m = 2 * mp + i
                    nc.scalar.activation(out=c1p[:, i, :], in_=fp2[:, i, 0:GRP],
                                         func=AF.Relu, bias=fc1bP[:, m:m + 1],
                                         scale=1.0 / 256.0)
                w1 = strU.tile([128, 2, GRP], BF, name="w1", tag="w1")
                nc.vector.tensor_scalar(out=w1[:], in0=c1p[:],
                                        scalar1=256.0, scalar2=-8.0,
                                        op0=ALU.mult, op1=ALU.add)
                for i in range(2):
                    m = 2 * mp + i
                    nc.vector.scalar_tensor_tensor(
                        out=ring[:, m, 4 * g + 1:4 * g + 5, 1:RW - 1],
                        in0=c1p[:, i, :].rearrange("p (r q) -> p r q", r=4),
                        scalar=16.0 / 256.0,
                        in1=w1[:, i, :].rearrange("p (r q) -> p r q", r=4),
                        op0=ALU.min, op1=ALU.mult)

            def emit_dw_pair(g, mp):
                # depthwise 3x3 for channel chunks 2mp, 2mp+1 of group g.
                # The flat ring holds rows 4g..4g+5 contiguously with zero
                # pads, so every group is uniform: per di one DoubleRow pair
                # (dj=-1/+1), plus one pair for the dj=0 taps of di=+-1,
                # plus one single for the center tap.  The conv bias rides
                # the c2p Relu bias port (emit_t2).
                dw2 = psF2.tile([128, 2, C], FP, name="dw2", tag="pf")
                for i in range(2):
                    m = 2 * mp + i
                    dw3 = dw2[:, i, 0:GRP].rearrange("p (r j) -> p r j", r=4)
                    mms = []
                    # center tap (di=0, dj=0): full range, starts accumulation
                    base_c = (ring_ap.offset + m * NRING * RW
                              + (4 * g + 1) * RW + 1)
                    sing_rhs = bass.AP(tensor=ring_ap.tensor, offset=base_c,
                                       ap=[[pstr_ring, 128], [RW, 4], [1, 96]])
                    mms.append((diags[:, m], sing_rhs, dw3[:, 0:4, :], None))
                    # dj=+-1 pairs for each di
                    for d_i, di in enumerate((-1, 0, 1)):
                        base = (ring_ap.offset + m * NRING * RW
                                + (4 * g + 1 + di) * RW)
                        if DW3D:
                            for r in range(4):
                                prr = bass.AP(
                                    tensor=ring_ap.tensor, offset=base + r * RW,
                                    ap=[[pstr_ring, 128], [2, 2], [1, 96]])
                                mms.append((diagp[:, m, d_i], prr,
                                            dw3[:, r:r + 1, :], DR))
                        else:
                            prr = bass.AP(
                                tensor=ring_ap.tensor, offset=base,
                                ap=[[pstr_ring, 128], [2, 2], [RW, 4], [1, 96]])
                            mms.append((diagp[:, m, d_i], prr,
                                        dw3[:, 0:4, :], DR))
                    # dj=0 taps of di=-1 and di=+1 as one DoubleRow pair
                    # (the two windows are 2 rows apart)
                    base0 = ring_ap.offset + m * NRING * RW + 4 * g * RW + 1
                    if DW3D:
                        for r in range(4):
                            prr = bass.AP(
                                tensor=ring_ap.tensor, offset=base0 + r * RW,
                                ap=[[pstr_ring, 128], [2 * RW, 2], [1, 96]])
                            mms.append((diagv[:, m], prr,
                                        dw3[:, r:r + 1, :], DR))
                    else:
                        prr = bass.AP(
                            tensor=ring_ap.tensor, offset=base0,
                            ap=[[pstr_ring, 128], [2 * RW, 2], [RW, 4],
                                [1, 96]])
                        mms.append((diagv[:, m], prr, dw3[:, 0:4, :], DR))
                    nmm = len(mms)
                    for k, (lhsT, rhs, o_ap, pm) in enumerate(mms):
                        nc.tensor.matmul(o_ap, lhsT, rhs,
                                         start=(k == 0), stop=(k == nmm - 1),
                                         perf_mode=pm)
                c2p = strE.tile([128, 2, GRP], BF, name="c2p", tag="c2p")
                for i in range(2):
                    m = 2 * mp + i
                    nc.scalar.activation(out=c2p[:, i, :], in_=dw2[:, i, 0:GRP],
                                         func=AF.Relu, bias=convbP[:, m:m + 1],
                                         scale=1.0 / 256.0)
                w2 = strE.tile([128, 2, GRP], BF, name="w2", tag="w2")
                nc.vector.tensor_scalar(out=w2[:], in0=c2p[:],
                                        scalar1=256.0, scalar2=-8.0,
                                        op0=ALU.mult, op1=ALU.add)
                t2p = ring_t2.tile([128, 2, GRP], F8, name=f"t2_{mp}", tag="t2")
                t2_of[g].append(t2p)
                nc.vector.scalar_tensor_tensor(
                    out=t2p[:], in0=c2p[:], scalar=16.0 / 256.0,
                    in1=w2[:], op0=ALU.min, op1=ALU.mult)

            def emit_fc2_group(g):
                g0 = g * GRP
                for sub in range(3):
                    r0 = g0 + sub * 128
                    op = psO.tile([128, C], FP, name="op", tag="op")
                    for j in range(MCH // 2):
                        nc.tensor.matmul(
                            op[:],
                            t2_of[g][j][:, :, sub * 128:(sub + 1) * 128],
                            fc2p[j][:], start=(j == 0), stop=False,
                            perf_mode=DR)
                    nc.tensor.matmul(op[:], onesP[0:1, 0:128], fc2b_sb[:],
                                     start=False, stop=True)
                    x2t = strC.tile([128, C], BF, name="x2t", tag="xt2")
                    nc.sync.dma_start(out=x2t[:], in_=x2d[r0:r0 + 128, :])
                    osb = strC.tile([128, C], BF, name="osb", tag="osb")
                    nc.vector.scalar_tensor_tensor(
                        out=osb[:], in0=op[:], scalar=1.0 / W16,
                        in1=x2t[:], op0=ALU.mult, op1=ALU.add)
                    nc.sync.dma_start(out=out_dram[r0:r0 + 128, :], in_=osb[:])
                del t2_of[g]

            # pipeline: attn(g) | fc1(g-1) | dw(g-2) | fc2(g-2) -- a full
            # group of dense matmul work is always ready while the serial
            # attention chain of the current group progresses.
            xh2T_of = {}
            for g in range(NGRP):
                xh2T_of[g] = emit_attn_group(g)
                t2_of.setdefault(g - 2, [])
                if g >= 1:
                    for mp in range(MCH // 2):
                        emit_fc1_pair(g - 1, xh2T_of[g - 1], mp)
                        if g >= 2:
                            emit_dw_pair(g - 2, mp)
                    del xh2T_of[g - 1]
                if g >= 2:
                    emit_fc2_group(g - 2)
            # epilogue
            gl = NGRP - 1
            t2_of.setdefault(gl - 1, [])
            for mp in range(MCH // 2):
                emit_fc1_pair(gl, xh2T_of[gl], mp)
                emit_dw_pair(gl - 1, mp)
            emit_fc2_group(gl - 1)
            t2_of.setdefault(gl, [])
            for mp in range(MCH // 2):
                emit_dw_pair(gl, mp)
            emit_fc2_group(gl)


def _host_prep(inputs):
    x = np.asarray(inputs["x"], np.float32)
    g1 = np.asarray(inputs["norm1_g"], np.float32)
    b1 = np.asarray(inputs["norm1_b"], np.float32)
    q_w = np.asarray(inputs["q_w"], np.float32)
    kv_w = np.asarray(inputs["kv_w"], np.float32)
    ag = np.asarray(inputs["attn_norm_g"], np.float32)
    ab = np.asarray(inputs["attn_norm_b"], np.float32)
    proj_w = np.asarray(inputs["proj_w"], np.float32)
    proj_b = np.asarray(inputs["proj_b"], np.float32)
    dconv_w = np.asarray(inputs["dconv_w"], np.float32)
    dconv_b = np.asarray(inputs["dconv_b"], np.float32)
    g2 = np.asarray(inputs["norm2_g"], np.float32)
    b2 = np.asarray(inputs["norm2_b"], np.float32)
    fc1_w = np.asarray(inputs["fc1_w"], np.float32)
    fc1_b = np.asarray(inputs["fc1_b"], np.float32)
    conv_w = np.asarray(inputs["conv_w"], np.float32)
    conv_b = np.asarray(inputs["conv_b"], np.float32)
    fc2_w = np.asarray(inputs["fc2_w"], np.float32)
    fc2_b = np.asarray(inputs["fc2_b"], np.float32)

    M = _build_masks()
    f8 = ml_dtypes.float8_e4m3fn
    bf = ml_dtypes.bfloat16

    qgw = (q_w * g1[None, :]) * SCALE
    qgwT = np.ascontiguousarray(qgw.T).astype(bf)
    qb = np.ascontiguousarray(((q_w @ b1) * SCALE).reshape(CCH, 128).T)
    kwT = np.ascontiguousarray(kv_w[0:C].T).astype(bf)
    vwT = np.ascontiguousarray(kv_w[C:2 * C].T).astype(bf)
    # projT: per head [65, C]; row 64 = proj_b/NH (rides the normalized
    # denominator row of A, which is exactly 1)
    projT = np.zeros((NH, 65, C), np.float32)
    projT[:, 0:HD, :] = proj_w.T.reshape(NH, HD, C)
    projT[:, HD, :] = proj_b[None, :] / NH
    projT = projT.astype(bf)
    # fc1: (w*g2/6).T  -> pairs of 128-row K chunks, x16, fp8
    fc1T6 = np.ascontiguousarray(((fc1_w * g2[None, :]) / 6.0).T)  # [C, HID]
    fc1p = np.zeros((2, 128, 2, HID), np.float32)
    for j in range(2):
        fc1p[j, :, 0] = fc1T6[(2 * j) * 128:(2 * j + 1) * 128]
        fc1p[j, :, 1] = fc1T6[(2 * j + 1) * 128:(2 * j + 2) * 128]
    fc1p = (fc1p * W16).astype(f8)
    # fc1 bias for the c1p Relu bias port: (W16*(b1 + W1@b2)/6 + 8)/256
    fc1bv = (W16 * (fc1_b + fc1_w @ b2) / 6.0 + 8.0) / 256.0
    fc1bP = np.ascontiguousarray(fc1bv.reshape(MCH, 128).T)
    # dwconv diagonals: pairs (dj=-1, dj=+1) and singles (dj=0), x16, fp8
    diagp = np.zeros((128, MCH, 3, 2, 128), np.float32)
    diags = np.zeros((128, MCH, 128), np.float32)
    for m in range(MCH):
        for d_i, di in enumerate((-1, 0, 1)):
            wm = conv_w[m * 128:(m + 1) * 128, 0]
            np.fill_diagonal(diagp[:, m, d_i, 0], wm[:, di + 1, 0])
            np.fill_diagonal(diagp[:, m, d_i, 1], wm[:, di + 1, 2])
        np.fill_diagonal(diags[:, m], conv_w[m * 128:(m + 1) * 128, 0][:, 1, 1])
    diagv = np.zeros((128, MCH, 2, 128), np.float32)
    for m in range(MCH):
        wm = conv_w[m * 128:(m + 1) * 128, 0]
        np.fill_diagonal(diagv[:, m, 0], wm[:, 0, 1])
        np.fill_diagonal(diagv[:, m, 1], wm[:, 2, 1])
    diagp = (diagp * W16).astype(f8)
    diags = (diags * W16).astype(f8)
    diagv = (diagv * W16).astype(f8)
    convbv = (W16 * conv_b / 6.0 + 8.0) / 256.0
    convbP = np.ascontiguousarray(convbv.reshape(MCH, 128).T)
    # fc2: (w*6).T -> pairs of 128-row K chunks, x16, fp8
    fc2T6 = np.ascontiguousarray((fc2_w * 6.0).T)  # [HID, C]
    fc2p = np.zeros((MCH // 2, 128, 2, C), np.float32)
    for j in range(MCH // 2):
        fc2p[j, :, 0] = fc2T6[(2 * j) * 128:(2 * j + 1) * 128]
        fc2p[j, :, 1] = fc2T6[(2 * j + 1) * 128:(2 * j + 2) * 128]
    fc2p = (fc2p * W16).astype(f8)
    fc2b = (fc2_b * W16).reshape(1, C).astype(bf)
    g1rep = np.broadcast_to(g1, (128, C)).astype(bf)
    abrep = np.stack([np.broadcast_to(AREAS[s] * b1, (128, C))
                      for s in range(4)], axis=1).astype(bf)
    agrep = np.broadcast_to(ag, (128, C)).astype(bf)
    btrep = np.broadcast_to(ab, (128, C)).astype(bf)
    poolw = np.zeros((128, CCH, 4, 9), np.float32)
    poolabd = np.zeros((128, CCH, 4), np.float32)
    for cc in range(CCH):
        for s in range(4):
            for tap in range(9):
                di, dj = TAPS[tap]
                poolw[:, cc, s, tap] = dconv_w[s, cc * 128:(cc + 1) * 128, 0,
                                               di + 1, dj + 1]
            poolabd[:, cc, s] = AREAS[s] * dconv_b[s, cc * 128:(cc + 1) * 128]
    v2init = np.zeros((L, NH * 128), np.float32)
    for h in range(NH):
        v2init[:, h * 128 + 64:h * 128 + 128] = 1.0
    v2init = v2init.astype(bf)

    shared = dict(qgwT=qgwT, qb=qb, kwT=kwT, vwT=vwT, projT=projT,
                  fc1p=fc1p, fc1bP=fc1bP, diagp=diagp, diags=diags,
                  diagv=diagv, convbP=convbP, fc2p=fc2p, fc2b=fc2b,
                  g1rep=g1rep,
                  abrep=abrep, agrep=agrep, btrep=btrep, poolw=poolw,
                  poolabd=poolabd, v2init=v2init,
                  identb=np.eye(128, dtype=np.float32).astype(bf))

    perms = []
    for half in range(2):
        f0 = 0 if half == 0 else NTOK - TRNG
        perms.append(np.concatenate([np.arange(f0, f0 + TRNG),
                                     np.arange(0, f0),
                                     np.arange(f0 + TRNG, NTOK)]))
    masks_p = [np.ascontiguousarray(M[p]).astype(f8) for p in perms]

    in_maps = []
    for b in range(B):
        for half in range(2):
            m = dict(shared)
            m["xbf"] = np.ascontiguousarray(x[b][perms[half]]).astype(bf)
            m["masks"] = masks_p[half]
            in_maps.append(m)
    return in_maps


def kernel(**inputs):
    if "nc" not in _cache:
        _cache["nc"] = build_program()
    nc = _cache["nc"]

    from concourse.bass_utils import run_bass_kernel_spmd

    in_maps = _host_prep(inputs)
    core_ids = list(range(8))
    res = run_bass_kernel_spmd(nc, in_maps, core_ids)

    x = np.asarray(inputs["x"], np.float32)
    out = np.empty_like(x)
    for b in range(B):
        o0 = np.asarray(res.results[2 * b]["out"], np.float32)
        o1 = np.asarray(res.results[2 * b + 1]["out"], np.float32)
        out[b, 0:HALF] = o0[0:HALF]
        out[b, HALF:] = o1[HALO:]
    return out
